# revision 3
# baseline (speedup 1.0000x reference)
"""Trainium2 Bass kernel for nn_AdaptivePatchEmbedding — optimized bf16.

Reference computes, over a [3,1024,1024] image:
  e0: 16x16 patches -> flatten -> @ Wb + b                    (8192 patches)
  e1: 32x32 patches -> bilinear-resize to 16x16 -> @ Wb + b   (4096 patches)
  e2: 64x64 patches -> bilinear-resize to 16x16 -> @ Wb + b   (2048 patches)
plus a ControlNet zero-init MLP branch on e1/e2 that is exactly zero for the
zero mlp weights (host numpy fallback keeps correctness otherwise).

Identities: 16x16/stride-16 conv == flatten+matmul with Wb=base_w.reshape(
D,-1).T; bilinear 32->16 == 2x2 block means; bilinear 64->16 == mean of the
2x2 block at rows {4i+1,4i+2} x cols {4j+1,4j+2}.  The host gathers,
pre-averages and k-major-transposes all patches so each core runs a pure
[1792,768] @ [768,768] bf16 matmul (14 jobs of 128 patches; ~27us of PE).

Pipeline design, from measured hardware behavior:
  - All DMA completions drain through one shared completion pipeline
    (~0.65us per ~196KB transfer, first exit ~3us after issue), ordered by
    issue.  The global issue order therefore equals the consumption order:
    jobs 0-2's leading halves on Sync/Scalar (6 HWDGE transfers, under the
    8 shared hardware queues so no slot-recycling stalls), W k-tiles 0-5 on
    GpSimd, then the remaining 11 xin jobs on GpSimd, then output DMAs.
  - ~196KB chunks are the sweet spot: the pipeline has a per-transfer cost,
    so finer slicing delays later exits; coarser chunks delay the first.
  - Jobs 0-2 interleave kt-major over the W k-tile arrivals (3 matmul-pairs
    per k-tile ~ arrival rate); jobs 3-13 run back-to-back.
  - 512-column warm matmuls bridge the PE from engine-open (~7.4us) to the
    first data exit (~11us) and start the ~6us clock ramp early.
  - One [128,768] bf16 output DMA per job on GpSimd (f32->bf16 conversion
    happens in the Scalar(448)/DVE(320) PSUM->SBUF copies); the last job's
    output splits into two half-DMAs on GpSimd+Sync to trim the tail.
"""

import os
import sys

for _p in ("/opt/trn_rl_repo", "/root/.axon_site/_ro/trn_rl_repo"):
    if os.path.isdir(_p) and _p not in sys.path:
        sys.path.insert(0, _p)

import numpy as np
import ml_dtypes

BF16 = ml_dtypes.bfloat16

C = 3
H = W = 1024
D = 768
BASE = 16
N0, N1, N2 = 8192, 4096, 2048
NCORES = 8
P0, P1, P2 = N0 // NCORES, N1 // NCORES, N2 // NCORES  # 1024, 512, 256
G0, G1, G2 = P0 // 128, P1 // 128, P2 // 128  # 8, 4, 2 jobs of 128 patches
NJOBS = G0 + G1 + G2  # 14
NKT = 6
NWARM = 7
NS = 448   # PSUM bank split for the accumulators
NC2 = 384  # Scalar/DVE output-copy split

_COMPILED = None


def _gather_host(image, coords0, coords1, coords2):
    """Gather + pre-average all patches into [N, 768] bf16 (per level)."""
    imgT = np.ascontiguousarray(image.transpose(1, 2, 0))  # [H, W, C] f32
    r16 = np.arange(16)
    j16 = np.arange(16)

    y, x = coords0[:, 0], coords0[:, 1]
    m = imgT[y[:, None, None] + r16[None, :, None],
             x[:, None, None] + j16[None, None, :]]  # [N, r, j, c]
    x0 = m.transpose(0, 3, 1, 2).reshape(-1, D)

    # row-pair sums [1023, 1024, 3]
    e1r = imgT[:-1] + imgT[1:]
    # col-pair sums at the two x-phases -> [2, 1023, 512, 3]
    e1rc = np.zeros((2, H - 1, W // 2, C), np.float32)
    e1rc[0] = e1r[:, 0::2] + e1r[:, 1::2]
    e1rc[1, :, :511] = e1r[:, 1:-1:2] + e1r[:, 2::2]

    y, x = coords1[:, 0], coords1[:, 1]
    p = x & 1
    x2 = (x - p) >> 1
    m = e1rc[p[:, None, None],
             y[:, None, None] + 2 * r16[None, :, None],
             x2[:, None, None] + j16[None, None, :]]  # [N, r, j, c]
    x1 = 0.25 * m.transpose(0, 3, 1, 2).reshape(-1, D)

    y, x = coords2[:, 0], coords2[:, 1]
    p = (x + 1) & 1
    x2 = (x + 1 - p) >> 1
    m = e1rc[p[:, None, None],
             (y + 1)[:, None, None] + 4 * r16[None, :, None],
             x2[:, None, None] + 2 * j16[None, None, :]]
    x2g = 0.25 * m.transpose(0, 3, 1, 2).reshape(-1, D)

    return x0.astype(BF16), x1.astype(BF16), x2g.astype(BF16)


def _build_graph():
    import concourse.bass as bass
    import concourse.mybir as mybir
    from concourse import bacc
    import concourse.tile as tile

    nc = bacc.Bacc("TRN2", target_bir_lowering=False, debug=False)
    f32 = mybir.dt.float32
    bf16 = mybir.dt.bfloat16

    xin_d = nc.dram_tensor("xin", [128, NJOBS, NKT, 128], bf16, kind="ExternalInput")
    w_d = nc.dram_tensor("wt", [128, NKT, D], bf16, kind="ExternalInput")
    out_d = nc.dram_tensor("out", [NJOBS * 128, D], bf16, kind="ExternalOutput")

    with tile.TileContext(nc) as tc:
        with (
            tc.tile_pool(name="static", bufs=1) as st,
            tc.tile_pool(name="xp", bufs=14) as xp,
            tc.tile_pool(name="psA", bufs=3, space="PSUM") as psA,
            tc.tile_pool(name="psW", bufs=2, space="PSUM") as psW,
            tc.tile_pool(name="outp", bufs=6) as outp,
        ):
            def in_job(g, eng=None):
                xt = xp.tile([128, NKT, 128], bf16, tag="xt")
                (eng or nc.gpsimd).dma_start(xt[:], xin_d[:, g])
                return xt

            # warmup operand first on GpSimd so the PE ramp starts as early
            # as the engine queues open
            ones = st.tile([128, 512], bf16, tag="ones")
            nc.gpsimd.memset(ones[:], 0.25)

            # All DMA completions drain through one shared ~300GB/s FIFO
            # whose first exit lands ~10us (preamble + issue + ~2.7us
            # latency), ordered by issue time.  So the global issue order
            # must equal the consumption order: W k-tiles + the first three
            # jobs' leading halves first (spread over the three engines so
            # they enter the FIFO together), then the remaining xin jobs,
            # then outputs.  HWDGE (Sync+Scalar) carries only 6 transfers
            # (under the 8 shared hardware queues); everything else rides
            # GpSimd so nothing can jump the FIFO ahead of the W stream.
            w_t = st.tile([128, NKT, D], bf16, tag="w")
            xt0 = xp.tile([128, NKT, 128], bf16, tag="xt")
            xt1 = xp.tile([128, NKT, 128], bf16, tag="xt")
            xt2 = xp.tile([128, NKT, 128], bf16, tag="xt")
            nc.sync.dma_start(xt0[:, 0:3], xin_d[:, 0, 0:3])
            nc.scalar.dma_start(xt1[:, 0:3], xin_d[:, 1, 0:3])
            nc.sync.dma_start(xt2[:, 0:3], xin_d[:, 2, 0:3])
            nc.scalar.dma_start(xt2[:, 3:NKT], xin_d[:, 2, 3:NKT])
            nc.sync.dma_start(xt0[:, 3:NKT], xin_d[:, 0, 3:NKT])
            nc.scalar.dma_start(xt1[:, 3:NKT], xin_d[:, 1, 3:NKT])
            for kt in range(NKT):
                nc.gpsimd.dma_start(w_t[:, kt], w_d[:, kt])
            pre = [xt0, xt1, xt2]
            # remaining jobs' inputs: issued up front on GpSimd right after
            # W; the xp pool holds every tile so no rotation wait can block
            # the issue stream
            tiles = {g: in_job(g) for g in range(3, NJOBS)}

            # Scalar act-table preload off the critical path
            warm_s = st.tile([128, 8], bf16, tag="warm")
            nc.scalar.copy(warm_s[:], ones[:, 0:8])
            # PE ramp bridging to the first W/X completions (~10.4us):
            # long 512-column warm matmuls amortize the per-instruction
            # semaphore overhead
            for _ in range(NWARM):
                wps = psW.tile([128, 512], f32, tag="wp")
                nc.tensor.matmul(wps[:], ones[:, 0:128], ones[:],
                                 start=True, stop=True)

            def job_mms(xt, acc0, acc1):
                for kt in range(NKT):
                    first, last = kt == 0, kt == NKT - 1
                    lhs = xt[:, kt, :]
                    nc.tensor.matmul(acc0[:], lhs, w_t[:, kt, 0:NS],
                                     start=first, stop=last)
                    nc.tensor.matmul(acc1[:], lhs, w_t[:, kt, NS:D],
                                     start=first, stop=last)

            def finish_job(acc0, acc1, g):
                o_t = outp.tile([128, D], bf16, tag="o")
                if g == NJOBS - 1:
                    # trim the tail: split copies and two half-DMAs
                    nc.scalar.copy(o_t[:, 0:NC2], acc0[:, 0:NC2])
                    nc.vector.tensor_copy(o_t[:, NC2:NS], acc0[:, NC2:NS])
                    nc.vector.tensor_copy(o_t[:, NS:D], acc1[:])
                    nc.gpsimd.dma_start(out_d[g * 128:(g + 1) * 128, 0:NC2],
                                        o_t[:, 0:NC2])
                    nc.sync.dma_start(out_d[g * 128:(g + 1) * 128, NC2:D],
                                      o_t[:, NC2:D])
                else:
                    nc.scalar.copy(o_t[:, 0:NS], acc0[:])
                    nc.vector.tensor_copy(o_t[:, NS:D], acc1[:])
                    nc.gpsimd.dma_start(out_d[g * 128:(g + 1) * 128, :], o_t[:])

            # jobs 0-2 interleaved kt-major over the 2-queue W arrivals
            accs = [(psA.tile([128, NS], f32, tag="acc0", name=f"ia0_{j}"),
                     psA.tile([128, D - NS], f32, tag="acc1", name=f"ia1_{j}"))
                    for j in range(3)]
            # jobs 0-2 interleaved kt-major, tracking the W k-tile arrivals
            for kt in range(NKT):
                first, last = kt == 0, kt == NKT - 1
                for j in range(3):
                    nc.tensor.matmul(accs[j][0][:], pre[j][:, kt, :],
                                     w_t[:, kt, 0:NS], start=first, stop=last)
                    nc.tensor.matmul(accs[j][1][:], pre[j][:, kt, :],
                                     w_t[:, kt, NS:D], start=first, stop=last)
            for j in range(3):
                finish_job(accs[j][0], accs[j][1], j)

            for g in range(3, NJOBS):
                acc0 = psA.tile([128, NS], f32, tag="acc0")
                acc1 = psA.tile([128, D - NS], f32, tag="acc1")
                job_mms(tiles[g], acc0, acc1)
                finish_job(acc0, acc1, g)

    nc.compile()
    return nc


def _get_compiled():
    global _COMPILED
    if _COMPILED is None:
        _COMPILED = _build_graph()
    return _COMPILED


def _mlp_correction(image, coords, g, agg_w, agg_b, mlp_w, mlp_b, base_w, base_b):
    """Host fallback: the zero-init-MLP branch, exact reference math."""
    Wb = base_w.reshape(D, -1).T
    ps = BASE * g
    n = coords.shape[0]
    patches = np.empty((n, C, ps, ps), np.float32)
    for k in range(n):
        y, x = int(coords[k, 0]), int(coords[k, 1])
        patches[k] = image[:, y:y + ps, x:x + ps]
    sub = patches.reshape(n, C, g, BASE, g, BASE).transpose(0, 2, 4, 1, 3, 5)
    sub_e = sub.reshape(n, g, g, C * BASE * BASE) @ Wb + base_b
    agg = np.einsum('nhwd,odhw->no', sub_e, agg_w) + agg_b
    return agg @ mlp_w.T + mlp_b


def build_in_maps(image, coords0, coords1, coords2, base_w, base_b):
    coords0 = np.asarray(coords0).astype(np.int64)
    coords1 = np.asarray(coords1).astype(np.int64)
    coords2 = np.asarray(coords2).astype(np.int64)
    x0, x1, x2 = _gather_host(image, coords0, coords1, coords2)

    Wb = base_w.reshape(D, -1).T  # [768 k, 768 n]
    wt_np = np.ascontiguousarray(
        Wb.reshape(NKT, 128, D).transpose(1, 0, 2)).astype(BF16)

    in_maps = []
    for k in range(NCORES):
        xc = np.concatenate([
            x0[k * P0:(k + 1) * P0],
            x1[k * P1:(k + 1) * P1],
            x2[k * P2:(k + 1) * P2],
        ], axis=0)  # [1792, 768]
        # xin[k', g, kt, p] = xc[g*128+p, kt*128+k']
        xin = np.ascontiguousarray(
            xc.reshape(NJOBS, 128, NKT, 128).transpose(3, 0, 2, 1))
        in_maps.append(dict(xin=xin, wt=wt_np))
    return in_maps


def kernel(image, coords0, coords1, coords2, base_w, base_b,
           agg_w1, agg_b1, agg_w2, agg_b2, mlp_w1, mlp_b1, mlp_w2, mlp_b2):
    from concourse.bass_utils import run_bass_kernel_spmd

    image = np.asarray(image, dtype=np.float32)
    base_w = np.asarray(base_w, dtype=np.float32)
    base_b = np.asarray(base_b, dtype=np.float32)

    nc = _get_compiled()
    in_maps = build_in_maps(image, coords0, coords1, coords2, base_w, base_b)

    res = run_bass_kernel_spmd(nc, in_maps, core_ids=list(range(NCORES)))
    outs = [np.asarray(res.results[k]["out"], dtype=np.float32) for k in range(NCORES)]

    e0 = np.concatenate([o[0:P0] for o in outs], axis=0) + base_b
    e1 = np.concatenate([o[P0:P0 + P1] for o in outs], axis=0) + base_b
    e2 = np.concatenate([o[P0 + P1:] for o in outs], axis=0) + base_b

    if np.any(mlp_w1) or np.any(mlp_b1):
        e1 = e1 + _mlp_correction(image, np.asarray(coords1), 2,
                                  np.asarray(agg_w1, np.float32), np.asarray(agg_b1, np.float32),
                                  np.asarray(mlp_w1, np.float32), np.asarray(mlp_b1, np.float32),
                                  base_w, base_b)
    if np.any(mlp_w2) or np.any(mlp_b2):
        e2 = e2 + _mlp_correction(image, np.asarray(coords2), 4,
                                  np.asarray(agg_w2, np.float32), np.asarray(agg_b2, np.float32),
                                  np.asarray(mlp_w2, np.float32), np.asarray(mlp_b2, np.float32),
                                  base_w, base_b)

    return np.concatenate([e0, e1, e2], axis=0)


# revision 4
# speedup vs baseline: 1.0053x; 1.0053x over previous
"""Trainium2 Bass kernel for nn_AdaptivePatchEmbedding — optimized bf16.

Reference computes, over a [3,1024,1024] image:
  e0: 16x16 patches -> flatten -> @ Wb + b                    (8192 patches)
  e1: 32x32 patches -> bilinear-resize to 16x16 -> @ Wb + b   (4096 patches)
  e2: 64x64 patches -> bilinear-resize to 16x16 -> @ Wb + b   (2048 patches)
plus a ControlNet zero-init MLP branch on e1/e2 that is exactly zero for the
zero mlp weights (host numpy fallback keeps correctness otherwise).

Identities: 16x16/stride-16 conv == flatten+matmul with Wb=base_w.reshape(
D,-1).T; bilinear 32->16 == 2x2 block means; bilinear 64->16 == mean of the
2x2 block at rows {4i+1,4i+2} x cols {4j+1,4j+2}.  The host gathers,
pre-averages and k-major-transposes all patches so each core runs a pure
[1792,768] @ [768,768] bf16 matmul (14 jobs of 128 patches; ~27us of PE).

Pipeline design, from measured hardware behavior:
  - All DMA completions drain through one shared completion pipeline
    (~0.65us per ~196KB transfer, first exit ~3us after issue), ordered by
    issue.  The global issue order therefore equals the consumption order:
    jobs 0-2's leading halves on Sync/Scalar (6 HWDGE transfers, under the
    8 shared hardware queues so no slot-recycling stalls), W k-tiles 0-5 on
    GpSimd, then the remaining 11 xin jobs on GpSimd, then output DMAs.
  - ~196KB chunks are the sweet spot: the pipeline has a per-transfer cost,
    so finer slicing delays later exits; coarser chunks delay the first.
  - Jobs 0-2 interleave kt-major over the W k-tile arrivals (3 matmul-pairs
    per k-tile ~ arrival rate); jobs 3-13 run back-to-back.
  - 512-column warm matmuls bridge the PE from engine-open (~7.4us) to the
    first data exit (~11us) and start the ~6us clock ramp early.
  - One [128,768] bf16 output DMA per job on GpSimd (f32->bf16 conversion
    happens in the Scalar(448)/DVE(320) PSUM->SBUF copies); the last job's
    output splits into two half-DMAs on GpSimd+Sync to trim the tail.
"""

import os
import sys

for _p in ("/opt/trn_rl_repo", "/root/.axon_site/_ro/trn_rl_repo"):
    if os.path.isdir(_p) and _p not in sys.path:
        sys.path.insert(0, _p)

import numpy as np
import ml_dtypes

BF16 = ml_dtypes.bfloat16

C = 3
H = W = 1024
D = 768
BASE = 16
N0, N1, N2 = 8192, 4096, 2048
NCORES = 8
P0, P1, P2 = N0 // NCORES, N1 // NCORES, N2 // NCORES  # 1024, 512, 256
G0, G1, G2 = P0 // 128, P1 // 128, P2 // 128  # 8, 4, 2 jobs of 128 patches
NJOBS = G0 + G1 + G2  # 14
NKT = 6
NWARM = 7
NS = 448   # PSUM bank split for the accumulators
NC2 = 384  # Scalar/DVE output-copy split

_COMPILED = None


def _gather_host(image, coords0, coords1, coords2):
    """Gather + pre-average all patches into [N, 768] bf16 (per level)."""
    imgT = np.ascontiguousarray(image.transpose(1, 2, 0))  # [H, W, C] f32
    r16 = np.arange(16)
    j16 = np.arange(16)

    y, x = coords0[:, 0], coords0[:, 1]
    m = imgT[y[:, None, None] + r16[None, :, None],
             x[:, None, None] + j16[None, None, :]]  # [N, r, j, c]
    x0 = m.transpose(0, 3, 1, 2).reshape(-1, D)

    # row-pair sums [1023, 1024, 3]
    e1r = imgT[:-1] + imgT[1:]
    # col-pair sums at the two x-phases -> [2, 1023, 512, 3]
    e1rc = np.zeros((2, H - 1, W // 2, C), np.float32)
    e1rc[0] = e1r[:, 0::2] + e1r[:, 1::2]
    e1rc[1, :, :511] = e1r[:, 1:-1:2] + e1r[:, 2::2]

    y, x = coords1[:, 0], coords1[:, 1]
    p = x & 1
    x2 = (x - p) >> 1
    m = e1rc[p[:, None, None],
             y[:, None, None] + 2 * r16[None, :, None],
             x2[:, None, None] + j16[None, None, :]]  # [N, r, j, c]
    x1 = 0.25 * m.transpose(0, 3, 1, 2).reshape(-1, D)

    y, x = coords2[:, 0], coords2[:, 1]
    p = (x + 1) & 1
    x2 = (x + 1 - p) >> 1
    m = e1rc[p[:, None, None],
             (y + 1)[:, None, None] + 4 * r16[None, :, None],
             x2[:, None, None] + 2 * j16[None, None, :]]
    x2g = 0.25 * m.transpose(0, 3, 1, 2).reshape(-1, D)

    return x0.astype(BF16), x1.astype(BF16), x2g.astype(BF16)


def _build_graph():
    import concourse.bass as bass
    import concourse.mybir as mybir
    from concourse import bacc
    import concourse.tile as tile

    nc = bacc.Bacc("TRN2", target_bir_lowering=False, debug=False)
    f32 = mybir.dt.float32
    bf16 = mybir.dt.bfloat16

    xin_d = nc.dram_tensor("xin", [128, NJOBS, NKT, 128], bf16, kind="ExternalInput")
    w_d = nc.dram_tensor("wt", [128, NKT, D], bf16, kind="ExternalInput")
    out_d = nc.dram_tensor("out", [NJOBS * 128, D], bf16, kind="ExternalOutput")

    with tile.TileContext(nc) as tc:
        with (
            tc.tile_pool(name="static", bufs=1) as st,
            tc.tile_pool(name="xp", bufs=14) as xp,
            tc.tile_pool(name="psA", bufs=3, space="PSUM") as psA,
            tc.tile_pool(name="psW", bufs=2, space="PSUM") as psW,
            tc.tile_pool(name="outp", bufs=6) as outp,
        ):
            def in_job(g, eng=None):
                xt = xp.tile([128, NKT, 128], bf16, tag="xt")
                (eng or nc.gpsimd).dma_start(xt[:], xin_d[:, g])
                return xt

            # warmup operand: a small slice memsets first on GpSimd (96ns,
            # ahead of its DMA issues) so the PE ramp starts as early as the
            # engine queues open; DVE fills the rest in parallel for the
            # long warms
            ones = st.tile([128, 512], bf16, tag="ones")
            nc.gpsimd.memset(ones[:, 0:128], 0.25)
            nc.vector.memset(ones[:, 128:512], 0.25)

            # All DMA completions drain through one shared ~300GB/s FIFO
            # whose first exit lands ~10us (preamble + issue + ~2.7us
            # latency), ordered by issue time.  So the global issue order
            # must equal the consumption order: W k-tiles + the first three
            # jobs' leading halves first (spread over the three engines so
            # they enter the FIFO together), then the remaining xin jobs,
            # then outputs.  HWDGE (Sync+Scalar) carries only 6 transfers
            # (under the 8 shared hardware queues); everything else rides
            # GpSimd so nothing can jump the FIFO ahead of the W stream.
            w_t = st.tile([128, NKT, D], bf16, tag="w")
            xt0 = xp.tile([128, NKT, 128], bf16, tag="xt")
            xt1 = xp.tile([128, NKT, 128], bf16, tag="xt")
            xt2 = xp.tile([128, NKT, 128], bf16, tag="xt")
            nc.sync.dma_start(xt0[:, 0:3], xin_d[:, 0, 0:3])
            nc.scalar.dma_start(xt1[:, 0:3], xin_d[:, 1, 0:3])
            nc.sync.dma_start(xt2[:, 0:3], xin_d[:, 2, 0:3])
            nc.scalar.dma_start(xt2[:, 3:NKT], xin_d[:, 2, 3:NKT])
            nc.sync.dma_start(xt0[:, 3:NKT], xin_d[:, 0, 3:NKT])
            nc.scalar.dma_start(xt1[:, 3:NKT], xin_d[:, 1, 3:NKT])
            for kt in range(NKT):
                nc.gpsimd.dma_start(w_t[:, kt], w_d[:, kt])
            pre = [xt0, xt1, xt2]
            # remaining jobs' inputs: issued up front on GpSimd right after
            # W; the xp pool holds every tile so no rotation wait can block
            # the issue stream
            tiles = {g: in_job(g) for g in range(3, NJOBS)}

            # Scalar act-table preload off the critical path
            warm_s = st.tile([128, 8], bf16, tag="warm")
            nc.scalar.copy(warm_s[:], ones[:, 0:8])
            # PE ramp bridging to the first W/X completions (~11us): two
            # short warms start the clock ramp as soon as the small memset
            # lands, then long 512-column warms (which amortize the
            # per-instruction semaphore overhead) carry it to data arrival
            for _ in range(2):
                wps = psW.tile([128, 512], f32, tag="wp")
                nc.tensor.matmul(wps[:, 0:128], ones[:, 0:128], ones[:, 0:128],
                                 start=True, stop=True)
            for _ in range(NWARM):
                wps = psW.tile([128, 512], f32, tag="wp")
                nc.tensor.matmul(wps[:], ones[:, 0:128], ones[:],
                                 start=True, stop=True)

            def job_mms(xt, acc0, acc1):
                for kt in range(NKT):
                    first, last = kt == 0, kt == NKT - 1
                    lhs = xt[:, kt, :]
                    nc.tensor.matmul(acc0[:], lhs, w_t[:, kt, 0:NS],
                                     start=first, stop=last)
                    nc.tensor.matmul(acc1[:], lhs, w_t[:, kt, NS:D],
                                     start=first, stop=last)

            def finish_job(acc0, acc1, g):
                o_t = outp.tile([128, D], bf16, tag="o")
                if g == NJOBS - 1:
                    # trim the tail: split copies and two half-DMAs
                    nc.scalar.copy(o_t[:, 0:NC2], acc0[:, 0:NC2])
                    nc.vector.tensor_copy(o_t[:, NC2:NS], acc0[:, NC2:NS])
                    nc.vector.tensor_copy(o_t[:, NS:D], acc1[:])
                    nc.gpsimd.dma_start(out_d[g * 128:(g + 1) * 128, 0:NC2],
                                        o_t[:, 0:NC2])
                    nc.sync.dma_start(out_d[g * 128:(g + 1) * 128, NC2:D],
                                      o_t[:, NC2:D])
                else:
                    nc.scalar.copy(o_t[:, 0:NS], acc0[:])
                    nc.vector.tensor_copy(o_t[:, NS:D], acc1[:])
                    nc.gpsimd.dma_start(out_d[g * 128:(g + 1) * 128, :], o_t[:])

            # jobs 0-2 interleaved kt-major over the 2-queue W arrivals
            accs = [(psA.tile([128, NS], f32, tag="acc0", name=f"ia0_{j}"),
                     psA.tile([128, D - NS], f32, tag="acc1", name=f"ia1_{j}"))
                    for j in range(3)]
            # jobs 0-2 interleaved kt-major, tracking the W k-tile arrivals
            for kt in range(NKT):
                first, last = kt == 0, kt == NKT - 1
                for j in range(3):
                    nc.tensor.matmul(accs[j][0][:], pre[j][:, kt, :],
                                     w_t[:, kt, 0:NS], start=first, stop=last)
                    nc.tensor.matmul(accs[j][1][:], pre[j][:, kt, :],
                                     w_t[:, kt, NS:D], start=first, stop=last)
            for j in range(3):
                finish_job(accs[j][0], accs[j][1], j)

            for g in range(3, NJOBS):
                acc0 = psA.tile([128, NS], f32, tag="acc0")
                acc1 = psA.tile([128, D - NS], f32, tag="acc1")
                job_mms(tiles[g], acc0, acc1)
                finish_job(acc0, acc1, g)

    nc.compile()
    return nc


def _get_compiled():
    global _COMPILED
    if _COMPILED is None:
        _COMPILED = _build_graph()
    return _COMPILED


def _mlp_correction(image, coords, g, agg_w, agg_b, mlp_w, mlp_b, base_w, base_b):
    """Host fallback: the zero-init-MLP branch, exact reference math."""
    Wb = base_w.reshape(D, -1).T
    ps = BASE * g
    n = coords.shape[0]
    patches = np.empty((n, C, ps, ps), np.float32)
    for k in range(n):
        y, x = int(coords[k, 0]), int(coords[k, 1])
        patches[k] = image[:, y:y + ps, x:x + ps]
    sub = patches.reshape(n, C, g, BASE, g, BASE).transpose(0, 2, 4, 1, 3, 5)
    sub_e = sub.reshape(n, g, g, C * BASE * BASE) @ Wb + base_b
    agg = np.einsum('nhwd,odhw->no', sub_e, agg_w) + agg_b
    return agg @ mlp_w.T + mlp_b


def build_in_maps(image, coords0, coords1, coords2, base_w, base_b):
    coords0 = np.asarray(coords0).astype(np.int64)
    coords1 = np.asarray(coords1).astype(np.int64)
    coords2 = np.asarray(coords2).astype(np.int64)
    x0, x1, x2 = _gather_host(image, coords0, coords1, coords2)

    Wb = base_w.reshape(D, -1).T  # [768 k, 768 n]
    wt_np = np.ascontiguousarray(
        Wb.reshape(NKT, 128, D).transpose(1, 0, 2)).astype(BF16)

    in_maps = []
    for k in range(NCORES):
        xc = np.concatenate([
            x0[k * P0:(k + 1) * P0],
            x1[k * P1:(k + 1) * P1],
            x2[k * P2:(k + 1) * P2],
        ], axis=0)  # [1792, 768]
        # xin[k', g, kt, p] = xc[g*128+p, kt*128+k']
        xin = np.ascontiguousarray(
            xc.reshape(NJOBS, 128, NKT, 128).transpose(3, 0, 2, 1))
        in_maps.append(dict(xin=xin, wt=wt_np))
    return in_maps


def kernel(image, coords0, coords1, coords2, base_w, base_b,
           agg_w1, agg_b1, agg_w2, agg_b2, mlp_w1, mlp_b1, mlp_w2, mlp_b2):
    from concourse.bass_utils import run_bass_kernel_spmd

    image = np.asarray(image, dtype=np.float32)
    base_w = np.asarray(base_w, dtype=np.float32)
    base_b = np.asarray(base_b, dtype=np.float32)

    nc = _get_compiled()
    in_maps = build_in_maps(image, coords0, coords1, coords2, base_w, base_b)

    res = run_bass_kernel_spmd(nc, in_maps, core_ids=list(range(NCORES)))
    outs = [np.asarray(res.results[k]["out"], dtype=np.float32) for k in range(NCORES)]

    e0 = np.concatenate([o[0:P0] for o in outs], axis=0) + base_b
    e1 = np.concatenate([o[P0:P0 + P1] for o in outs], axis=0) + base_b
    e2 = np.concatenate([o[P0 + P1:] for o in outs], axis=0) + base_b

    if np.any(mlp_w1) or np.any(mlp_b1):
        e1 = e1 + _mlp_correction(image, np.asarray(coords1), 2,
                                  np.asarray(agg_w1, np.float32), np.asarray(agg_b1, np.float32),
                                  np.asarray(mlp_w1, np.float32), np.asarray(mlp_b1, np.float32),
                                  base_w, base_b)
    if np.any(mlp_w2) or np.any(mlp_b2):
        e2 = e2 + _mlp_correction(image, np.asarray(coords2), 4,
                                  np.asarray(agg_w2, np.float32), np.asarray(agg_b2, np.float32),
                                  np.asarray(mlp_w2, np.float32), np.asarray(mlp_b2, np.float32),
                                  base_w, base_b)

    return np.concatenate([e0, e1, e2], axis=0)


# revision 5
# speedup vs baseline: 1.0134x; 1.0080x over previous
"""Trainium2 Bass kernel for nn_AdaptivePatchEmbedding — optimized bf16.

Reference computes, over a [3,1024,1024] image:
  e0: 16x16 patches -> flatten -> @ Wb + b                    (8192 patches)
  e1: 32x32 patches -> bilinear-resize to 16x16 -> @ Wb + b   (4096 patches)
  e2: 64x64 patches -> bilinear-resize to 16x16 -> @ Wb + b   (2048 patches)
plus a ControlNet zero-init MLP branch on e1/e2 that is exactly zero for the
zero mlp weights (host numpy fallback keeps correctness otherwise).

Identities: 16x16/stride-16 conv == flatten+matmul with Wb=base_w.reshape(
D,-1).T; bilinear 32->16 == 2x2 block means; bilinear 64->16 == mean of the
2x2 block at rows {4i+1,4i+2} x cols {4j+1,4j+2}.  The host gathers,
pre-averages and k-major-transposes all patches so each core runs a pure
[1792,768] @ [768,768] bf16 matmul (14 jobs of 128 patches; ~27us of PE).

Pipeline design, from measured hardware behavior:
  - All DMA completions drain through one shared completion pipeline
    (~0.65us per ~196KB transfer, first exit ~3us after issue), ordered by
    issue.  The global issue order therefore equals the consumption order:
    jobs 0-2's leading halves on Sync/Scalar (6 HWDGE transfers, under the
    8 shared hardware queues so no slot-recycling stalls), W k-tiles 0-5 on
    GpSimd, then the remaining 11 xin jobs on GpSimd, then output DMAs.
  - ~196KB chunks are the sweet spot: the pipeline has a per-transfer cost,
    so finer slicing delays later exits; coarser chunks delay the first.
  - Jobs 0-2 interleave kt-major over the W k-tile arrivals (3 matmul-pairs
    per k-tile ~ arrival rate); jobs 3-13 run back-to-back.
  - 512-column warm matmuls bridge the PE from engine-open (~7.4us) to the
    first data exit (~11us) and start the ~6us clock ramp early.
  - One [128,768] bf16 output DMA per job on GpSimd (f32->bf16 conversion
    happens in the Scalar(448)/DVE(320) PSUM->SBUF copies); the last job's
    output splits into two half-DMAs on GpSimd+Sync to trim the tail.
"""

import os
import sys

for _p in ("/opt/trn_rl_repo", "/root/.axon_site/_ro/trn_rl_repo"):
    if os.path.isdir(_p) and _p not in sys.path:
        sys.path.insert(0, _p)

import numpy as np
import ml_dtypes

BF16 = ml_dtypes.bfloat16

C = 3
H = W = 1024
D = 768
BASE = 16
N0, N1, N2 = 8192, 4096, 2048
NCORES = 8
P0, P1, P2 = N0 // NCORES, N1 // NCORES, N2 // NCORES  # 1024, 512, 256
G0, G1, G2 = P0 // 128, P1 // 128, P2 // 128  # 8, 4, 2 jobs of 128 patches
NJOBS = G0 + G1 + G2  # 14
NKT = 6
NWARM = 9
NS = 448   # PSUM bank split for the accumulators
NC2 = 384  # Scalar/DVE output-copy split

_COMPILED = None


def _gather_host(image, coords0, coords1, coords2):
    """Gather + pre-average all patches into [N, 768] bf16 (per level)."""
    imgT = np.ascontiguousarray(image.transpose(1, 2, 0))  # [H, W, C] f32
    r16 = np.arange(16)
    j16 = np.arange(16)

    y, x = coords0[:, 0], coords0[:, 1]
    m = imgT[y[:, None, None] + r16[None, :, None],
             x[:, None, None] + j16[None, None, :]]  # [N, r, j, c]
    x0 = m.transpose(0, 3, 1, 2).reshape(-1, D)

    # row-pair sums [1023, 1024, 3]
    e1r = imgT[:-1] + imgT[1:]
    # col-pair sums at the two x-phases -> [2, 1023, 512, 3]
    e1rc = np.zeros((2, H - 1, W // 2, C), np.float32)
    e1rc[0] = e1r[:, 0::2] + e1r[:, 1::2]
    e1rc[1, :, :511] = e1r[:, 1:-1:2] + e1r[:, 2::2]

    y, x = coords1[:, 0], coords1[:, 1]
    p = x & 1
    x2 = (x - p) >> 1
    m = e1rc[p[:, None, None],
             y[:, None, None] + 2 * r16[None, :, None],
             x2[:, None, None] + j16[None, None, :]]  # [N, r, j, c]
    x1 = 0.25 * m.transpose(0, 3, 1, 2).reshape(-1, D)

    y, x = coords2[:, 0], coords2[:, 1]
    p = (x + 1) & 1
    x2 = (x + 1 - p) >> 1
    m = e1rc[p[:, None, None],
             (y + 1)[:, None, None] + 4 * r16[None, :, None],
             x2[:, None, None] + 2 * j16[None, None, :]]
    x2g = 0.25 * m.transpose(0, 3, 1, 2).reshape(-1, D)

    return x0.astype(BF16), x1.astype(BF16), x2g.astype(BF16)


def _build_graph():
    import concourse.bass as bass
    import concourse.mybir as mybir
    from concourse import bacc
    import concourse.tile as tile

    nc = bacc.Bacc("TRN2", target_bir_lowering=False, debug=False)
    f32 = mybir.dt.float32
    bf16 = mybir.dt.bfloat16

    xin_d = nc.dram_tensor("xin", [128, NJOBS, NKT, 128], bf16, kind="ExternalInput")
    w_d = nc.dram_tensor("wt", [128, NKT, D], bf16, kind="ExternalInput")
    out_d = nc.dram_tensor("out", [NJOBS * 128, D], bf16, kind="ExternalOutput")

    with tile.TileContext(nc) as tc:
        with (
            tc.tile_pool(name="static", bufs=1) as st,
            tc.tile_pool(name="xp", bufs=14) as xp,
            tc.tile_pool(name="psA", bufs=3, space="PSUM") as psA,
            tc.tile_pool(name="psW", bufs=2, space="PSUM") as psW,
            tc.tile_pool(name="outp", bufs=6) as outp,
        ):
            def in_job(g, eng=None):
                xt = xp.tile([128, NKT, 128], bf16, tag="xt")
                (eng or nc.gpsimd).dma_start(xt[:], xin_d[:, g])
                return xt

            # warmup operand: a small slice memsets first on GpSimd (96ns,
            # ahead of its DMA issues) so the PE ramp starts as early as the
            # engine queues open; DVE fills the rest in parallel for the
            # long warms
            ones = st.tile([128, 512], bf16, tag="ones")
            nc.gpsimd.memset(ones[:, 0:128], 0.25)
            nc.vector.memset(ones[:, 128:512], 0.25)

            # All DMA completions drain through one shared ~300GB/s FIFO
            # whose first exit lands ~10us (preamble + issue + ~2.7us
            # latency), ordered by issue time.  So the global issue order
            # must equal the consumption order: W k-tiles + the first three
            # jobs' leading halves first (spread over the three engines so
            # they enter the FIFO together), then the remaining xin jobs,
            # then outputs.  HWDGE (Sync+Scalar) carries only 6 transfers
            # (under the 8 shared hardware queues); everything else rides
            # GpSimd so nothing can jump the FIFO ahead of the W stream.
            w_t = st.tile([128, NKT, D], bf16, tag="w")
            xt0 = xp.tile([128, NKT, 128], bf16, tag="xt")
            xt1 = xp.tile([128, NKT, 128], bf16, tag="xt")
            xt2 = xp.tile([128, NKT, 128], bf16, tag="xt")
            nc.sync.dma_start(xt0[:, 0:3], xin_d[:, 0, 0:3])
            nc.scalar.dma_start(xt1[:, 0:3], xin_d[:, 1, 0:3])
            nc.sync.dma_start(xt2[:, 0:3], xin_d[:, 2, 0:3])
            nc.scalar.dma_start(xt2[:, 3:NKT], xin_d[:, 2, 3:NKT])
            nc.sync.dma_start(xt0[:, 3:NKT], xin_d[:, 0, 3:NKT])
            nc.scalar.dma_start(xt1[:, 3:NKT], xin_d[:, 1, 3:NKT])
            for kt in range(NKT):
                nc.gpsimd.dma_start(w_t[:, kt], w_d[:, kt])
            pre = [xt0, xt1, xt2]
            # remaining jobs' inputs: issued up front on GpSimd right after
            # W; the xp pool holds every tile so no rotation wait can block
            # the issue stream
            tiles = {g: in_job(g) for g in range(3, NJOBS)}

            # Scalar act-table preload off the critical path
            warm_s = st.tile([128, 8], bf16, tag="warm")
            nc.scalar.copy(warm_s[:], ones[:, 0:8])
            # PE ramp bridging to the first W/X completions (~11us): two
            # short warms start the clock ramp as soon as the small memset
            # lands, then long 512-column warms (which amortize the
            # per-instruction semaphore overhead) carry it to data arrival
            for _ in range(2):
                wps = psW.tile([128, 512], f32, tag="wp")
                nc.tensor.matmul(wps[:, 0:128], ones[:, 0:128], ones[:, 0:128],
                                 start=True, stop=True)
            for _ in range(NWARM):
                wps = psW.tile([128, 512], f32, tag="wp")
                nc.tensor.matmul(wps[:], ones[:, 0:128], ones[:],
                                 start=True, stop=True)

            def job_mms(xt, acc0, acc1):
                for kt in range(NKT):
                    first, last = kt == 0, kt == NKT - 1
                    lhs = xt[:, kt, :]
                    nc.tensor.matmul(acc0[:], lhs, w_t[:, kt, 0:NS],
                                     start=first, stop=last)
                    nc.tensor.matmul(acc1[:], lhs, w_t[:, kt, NS:D],
                                     start=first, stop=last)

            def finish_job(acc0, acc1, g):
                o_t = outp.tile([128, D], bf16, tag="o")
                if g == NJOBS - 1:
                    # trim the tail: split copies and two half-DMAs
                    nc.scalar.copy(o_t[:, 0:NC2], acc0[:, 0:NC2])
                    nc.vector.tensor_copy(o_t[:, NC2:NS], acc0[:, NC2:NS])
                    nc.vector.tensor_copy(o_t[:, NS:D], acc1[:])
                    nc.gpsimd.dma_start(out_d[g * 128:(g + 1) * 128, 0:NC2],
                                        o_t[:, 0:NC2])
                    nc.sync.dma_start(out_d[g * 128:(g + 1) * 128, NC2:D],
                                      o_t[:, NC2:D])
                else:
                    nc.scalar.copy(o_t[:, 0:NS], acc0[:])
                    nc.vector.tensor_copy(o_t[:, NS:D], acc1[:])
                    nc.gpsimd.dma_start(out_d[g * 128:(g + 1) * 128, :], o_t[:])

            # jobs 0-2 interleaved kt-major over the 2-queue W arrivals
            accs = [(psA.tile([128, NS], f32, tag="acc0", name=f"ia0_{j}"),
                     psA.tile([128, D - NS], f32, tag="acc1", name=f"ia1_{j}"))
                    for j in range(3)]
            # jobs 0-2 interleaved kt-major, tracking the W k-tile arrivals
            for kt in range(NKT):
                first, last = kt == 0, kt == NKT - 1
                for j in range(3):
                    nc.tensor.matmul(accs[j][0][:], pre[j][:, kt, :],
                                     w_t[:, kt, 0:NS], start=first, stop=last)
                    nc.tensor.matmul(accs[j][1][:], pre[j][:, kt, :],
                                     w_t[:, kt, NS:D], start=first, stop=last)
            for j in range(3):
                finish_job(accs[j][0], accs[j][1], j)

            for g in range(3, NJOBS):
                acc0 = psA.tile([128, NS], f32, tag="acc0")
                acc1 = psA.tile([128, D - NS], f32, tag="acc1")
                job_mms(tiles[g], acc0, acc1)
                finish_job(acc0, acc1, g)

    nc.compile()
    return nc


def _get_compiled():
    global _COMPILED
    if _COMPILED is None:
        _COMPILED = _build_graph()
    return _COMPILED


def _mlp_correction(image, coords, g, agg_w, agg_b, mlp_w, mlp_b, base_w, base_b):
    """Host fallback: the zero-init-MLP branch, exact reference math."""
    Wb = base_w.reshape(D, -1).T
    ps = BASE * g
    n = coords.shape[0]
    patches = np.empty((n, C, ps, ps), np.float32)
    for k in range(n):
        y, x = int(coords[k, 0]), int(coords[k, 1])
        patches[k] = image[:, y:y + ps, x:x + ps]
    sub = patches.reshape(n, C, g, BASE, g, BASE).transpose(0, 2, 4, 1, 3, 5)
    sub_e = sub.reshape(n, g, g, C * BASE * BASE) @ Wb + base_b
    agg = np.einsum('nhwd,odhw->no', sub_e, agg_w) + agg_b
    return agg @ mlp_w.T + mlp_b


def build_in_maps(image, coords0, coords1, coords2, base_w, base_b):
    coords0 = np.asarray(coords0).astype(np.int64)
    coords1 = np.asarray(coords1).astype(np.int64)
    coords2 = np.asarray(coords2).astype(np.int64)
    x0, x1, x2 = _gather_host(image, coords0, coords1, coords2)

    Wb = base_w.reshape(D, -1).T  # [768 k, 768 n]
    wt_np = np.ascontiguousarray(
        Wb.reshape(NKT, 128, D).transpose(1, 0, 2)).astype(BF16)

    in_maps = []
    for k in range(NCORES):
        xc = np.concatenate([
            x0[k * P0:(k + 1) * P0],
            x1[k * P1:(k + 1) * P1],
            x2[k * P2:(k + 1) * P2],
        ], axis=0)  # [1792, 768]
        # xin[k', g, kt, p] = xc[g*128+p, kt*128+k']
        xin = np.ascontiguousarray(
            xc.reshape(NJOBS, 128, NKT, 128).transpose(3, 0, 2, 1))
        in_maps.append(dict(xin=xin, wt=wt_np))
    return in_maps


def kernel(image, coords0, coords1, coords2, base_w, base_b,
           agg_w1, agg_b1, agg_w2, agg_b2, mlp_w1, mlp_b1, mlp_w2, mlp_b2):
    from concourse.bass_utils import run_bass_kernel_spmd

    image = np.asarray(image, dtype=np.float32)
    base_w = np.asarray(base_w, dtype=np.float32)
    base_b = np.asarray(base_b, dtype=np.float32)

    nc = _get_compiled()
    in_maps = build_in_maps(image, coords0, coords1, coords2, base_w, base_b)

    res = run_bass_kernel_spmd(nc, in_maps, core_ids=list(range(NCORES)))
    outs = [np.asarray(res.results[k]["out"], dtype=np.float32) for k in range(NCORES)]

    e0 = np.concatenate([o[0:P0] for o in outs], axis=0) + base_b
    e1 = np.concatenate([o[P0:P0 + P1] for o in outs], axis=0) + base_b
    e2 = np.concatenate([o[P0 + P1:] for o in outs], axis=0) + base_b

    if np.any(mlp_w1) or np.any(mlp_b1):
        e1 = e1 + _mlp_correction(image, np.asarray(coords1), 2,
                                  np.asarray(agg_w1, np.float32), np.asarray(agg_b1, np.float32),
                                  np.asarray(mlp_w1, np.float32), np.asarray(mlp_b1, np.float32),
                                  base_w, base_b)
    if np.any(mlp_w2) or np.any(mlp_b2):
        e2 = e2 + _mlp_correction(image, np.asarray(coords2), 4,
                                  np.asarray(agg_w2, np.float32), np.asarray(agg_b2, np.float32),
                                  np.asarray(mlp_w2, np.float32), np.asarray(mlp_b2, np.float32),
                                  base_w, base_b)

    return np.concatenate([e0, e1, e2], axis=0)
